# revision 12
# baseline (speedup 1.0000x reference)
"""HGNN (2-layer hetero GraphSAGE + 8 heads) on 8 trn2 NeuronCores.

Single-launch design. Nodes are sharded dst-interleaved (core = v % 8,
local = v // 8); all src gather indices are pre-translated into the
"concatenated core blocks" order pos(v) = (v%8)*n_loc + v//8, which is the
layout produced by AllGather of per-core blocks. Both layers share one edge
packing (same graph), and the inter-layer halo exchange runs ON DEVICE:

  AllGather(x shards) -> tab0 -> layer1 -> PE-transpose -> AllGather -> tab1
  -> layer2 -> 8-head matmul -> yT

Features, weights and selection matrices are bf16 (fp32 PSUM accumulation);
the head bias is added host-side in fp32. All per-core inputs ship as ONE
consolidated tensor (~5.7MB/core vs ~170MB/core replicated in the two-launch
version): each named input costs ~60ms of PJRT-over-axon latency on top of
~21-50MB/s bandwidth, and the weight block itself is sharded 16 rows per
core and AllGathered on device.

Aggregation per 512-dst PSUM group: edges (dst-sorted) are cut into 128-edge
windows on a column grid uniform across cores, bucketed by src block of
25000 rows (int16 gather indices, 8x partition-replicated on device). Per
window one indirect DMA gathers 128 src rows; the 0/1 selection
sel[e, j] = (rel_dst[e] == j) is one int8 DVE is_equal; PE accumulates
g.T @ sel into PSUM, yielding the scatter-SUM s^T in [feat, dst]
orientation; the 1/cnt mean scaling is applied per dst column from scale
rows materialized on device by K=1 ones-outer-product matmuls.
"""
import time
import numpy as np
import ml_dtypes

import concourse.bass as bass
import concourse.bacc as bacc
import concourse.mybir as mybir
import concourse.tile as tile
from concourse.bass_utils import run_bass_kernel_spmd

P = 128
D = 128
NCORES = 8
GROUP = 512       # psum columns per accumulation group
S = 128           # max dst-column span per 128-edge window
BUCK = 25000      # src rows per int16 gather bucket
NB, NS = 100000, 50000
NLB, NLS = NB // NCORES, NS // NCORES   # 12500, 6250
BF16 = ml_dtypes.bfloat16


# ---------------------------------------------------------------- host prep
def _shard_edges(src, dst, n_dst):
    """Split edges by dst core; per core return (src, dst_local) dst-sorted.
    src must already be translated to concatenated-block order."""
    core = dst % NCORES
    loc = dst // NCORES
    out = []
    for c in range(NCORES):
        m = core == c
        s, d = src[m], loc[m]
        o = np.argsort(d, kind="stable")
        out.append((s[o].astype(np.int64), d[o].astype(np.int64)))
    return out


def _pack_type(per_core, n_loc, n_src):
    """Bucketed uniform-across-cores window packing for dma_gather.

    Returns (idx16 per bucket: list of [NCORES, 128, cols_b],
             rel [NCORES, P, Wtot] bf16, invc [NCORES, P, Wtot] bf16,
             groups: per group list of (bucket, k_local, col_off, span),
             gb_meta: per group dict bucket -> (idx_col_base, Nk))."""
    nbuck = (n_src + BUCK - 1) // BUCK
    ngroups = (n_loc + GROUP - 1) // GROUP
    pcb = [[None] * nbuck for _ in range(NCORES)]
    cumb = [[None] * nbuck for _ in range(NCORES)]
    counts_all = []
    for cc, (s, d) in enumerate(per_core):
        counts_all.append(np.bincount(d, minlength=n_loc))
        for b in range(nbuck):
            m = (s >= b * BUCK) & (s < (b + 1) * BUCK)
            sb_, db_ = s[m], d[m]
            pcb[cc][b] = (sb_ - b * BUCK, db_)
            cnt = np.bincount(db_, minlength=n_loc)
            cumb[cc][b] = np.concatenate([[0], np.cumsum(cnt)])
    invc_dst = [1.0 / np.maximum(c, 1) for c in counts_all]

    groups, gb_meta = [], []
    rel_cols = [[] for _ in range(NCORES)]
    idx_flat = [[[] for _ in range(nbuck)] for _ in range(NCORES)]
    idx_base = [0] * nbuck
    for g in range(ngroups):
        c0, c1 = g * GROUP, min((g + 1) * GROUP, n_loc)
        wins, meta = [], {}
        for b in range(nbuck):
            k_local = 0
            c = c0
            while c < c1:
                span = min(S, c1 - c)
                while span > 1:
                    ok = all(cumb[cc][b][c + span] - cumb[cc][b][c] <= P
                             for cc in range(NCORES))
                    if ok:
                        break
                    span -= 1
                for cc in range(NCORES):
                    s_arr, d_arr = pcb[cc][b]
                    a2, b2 = cumb[cc][b][c], cumb[cc][b][c + span]
                    n = b2 - a2
                    assert n <= P
                    icol = np.zeros(P, np.int16)
                    rcol = np.full(P, -1, np.int8)
                    icol[:n] = s_arr[a2:b2].astype(np.int16)
                    rcol[:n] = (d_arr[a2:b2] - c).astype(np.int8)
                    idx_flat[cc][b].append(icol)
                    rel_cols[cc].append(rcol)
                wins.append((b, k_local, c - c0, span))
                k_local += 1
                c += span
            if k_local:
                meta[b] = (idx_base[b], k_local * P)
                idx_base[b] += k_local * P
        groups.append(wins)
        gb_meta.append(meta)

    # int16 device layout per bucket: flat i at [i%16, i//16]; the 8x
    # partition replication dma_gather wants is done on device.
    idx16 = []
    for b in range(nbuck):
        per_core_arr = []
        for cc in range(NCORES):
            flat = (np.concatenate(idx_flat[cc][b]) if idx_flat[cc][b]
                    else np.zeros(16, np.int16))
            per_core_arr.append(flat.reshape(-1, 16).T)   # [16, cols]
        idx16.append(np.stack(per_core_arr).astype(np.int16))
    rel = np.stack([np.stack(cols, 1) for cols in rel_cols]).astype(np.int8)
    ivcd = np.stack(invc_dst).astype(BF16)    # [NCORES, n_loc] per-dst 1/cnt
    return idx16, rel, ivcd, groups, gb_meta


# ------------------------------------------------------------- device build
def _build(cfg):
    """Build the merged 2-layer SPMD program. cfg keys:
      types: dict name -> dict(tab ('b'|'s'), Wtot, groups, gb_meta, bcols)
      stage: 0=allgather only, 1=+L1 s-groups, 2=+ns1 allgather,
             3=+L1 b-groups + nb1 allgather, 4=full
    """
    STAGE = 4
    t_build0 = time.time()
    nc = bacc.Bacc("TRN2", target_bir_lowering=False, debug=False,
                   num_devices=NCORES)
    f32, bf16, i16 = mybir.dt.float32, mybir.dt.bfloat16, mybir.dt.int16
    types = cfg["types"]

    # ONE consolidated input tensor (each named input costs ~60ms of
    # PJRT-over-axon transfer latency). Layout, in 256B rows of [R, D] bf16:
    #   rows 0..NLB+NLS      : xb shard rows then xs shard rows
    #   then ivc [1, IVC] bf16: per-dst 1/cnt for bb | sb | bs (512-padded)
    #   then wbsh [16, NWB] bf16: this core's 16 partitions of the weights
    #        (the full [128, NWB] weight block is AllGathered on device)
    #   then aux [128, XB] u8: wts_f | rel per type | int8 iota
    #   then idx [16, IC] i16: per (type, bucket) column sections (IC % 128 == 0)
    NWB = 8 * D + 8 + S + D    # 8 mats | WhT | ones row | identity
    NWB += (-NWB) % 128        # 1408: 16-row shard must be whole 256B rows
    NLBP = NLB + (-NLB) % GROUP      # 12800
    NLSP = NLS + (-NLS) % GROUP      # 6400
    IVC = NLBP + 2 * NLSP            # bb | sb | bs scale vectors
    idx_off = {}
    col = 0
    for name, t in types.items():
        for b, cb in enumerate(t["bcols"]):
            w = max(cb, 16)
            idx_off[(name, b)] = (col, w)
            col += w
    col += (-col) % 128                           # 16*col*2 % 256 == 0
    IC = col
    aux_off = {}
    ab = 0
    aux_off["wf"] = ab; ab += 16                  # [128, 4] f32
    for name, t in types.items():
        aux_off[f"rel_{name}"] = ab; ab += t["Wtot"]   # int8
        ab += (-ab) % 4
    aux_off["iota"] = ab; ab += S                 # [128, S] int8
    ab += (-ab) % 256
    XB = ab
    XROWS = NLB + NLS
    VROWS = IVC * 2 // 256
    WROWS = 16 * NWB * 2 // 256      # = NWB // 8; NWB % 8 == 0
    AROWS = P * XB // 256
    IROWS = 16 * IC * 2 // 256
    d_all = nc.dram_tensor(
        "blob", [XROWS + VROWS + WROWS + AROWS + IROWS, D], bf16,
        kind="ExternalInput")
    d_x = d_all   # rows 0..XROWS
    d_ivc = (d_all[XROWS:XROWS + VROWS, :]
             .rearrange("(o k) b -> o (k b)", o=1))
    r0 = XROWS + VROWS
    d_wbsh = d_all[r0:r0 + WROWS, :]
    r0 += WROWS
    d_aux = (d_all[r0:r0 + AROWS, :].bitcast(mybir.dt.uint8)
             .rearrange("(p k) b -> p (k b)", p=P))
    d_idx_all = (d_all[r0 + AROWS:r0 + AROWS + IROWS, :]
                 .bitcast(mybir.dt.int16)
                 .rearrange("(p k) b -> p (k b)", p=16))
    d_yT = nc.dram_tensor("yT", [8, NLB], bf16, kind="ExternalOutput")
    d_dbg_b = d_dbg_s = None
    if STAGE == 5:   # debug: emit layer-1 outputs (block rows, bf16)
        d_dbg_b = nc.dram_tensor("dbg_b", [NLB, D], bf16, kind="ExternalOutput")
        d_dbg_s = nc.dram_tensor("dbg_s", [NLS, D], bf16, kind="ExternalOutput")

    from contextlib import ExitStack
    with tile.TileContext(nc) as tc, ExitStack() as ctx:
        dram = ctx.enter_context(tc.tile_pool(name="dram", bufs=1, space="DRAM"))
        wpool = ctx.enter_context(tc.tile_pool(name="w", bufs=1))
        ipool = ctx.enter_context(tc.tile_pool(name="i", bufs=1))
        kpool = ctx.enter_context(tc.tile_pool(name="k", bufs=1))
        gpool = ctx.enter_context(tc.tile_pool(name="g", bufs=4))
        selpool = ctx.enter_context(tc.tile_pool(name="sel", bufs=2))
        spool = ctx.enter_context(tc.tile_pool(name="s", bufs=3))
        appool = ctx.enter_context(tc.tile_pool(name="ap", bufs=2, space="PSUM"))
        s2pool = ctx.enter_context(tc.tile_pool(name="s2", bufs=2, space="PSUM"))
        tpool = ctx.enter_context(tc.tile_pool(name="t", bufs=1, space="PSUM"))

        # ---- DRAM scratch: bounce blocks + gathered tables
        bn_xb = dram.tile([NLB, D], bf16)
        bn_xs = dram.tile([NLS, D], bf16)
        tab_b0 = dram.tile([NB, D], bf16)
        tab_s0 = dram.tile([NS, D], bf16)
        bn_nb1 = dram.tile([NLB, D], bf16)
        bn_ns1 = dram.tile([NLS, D], bf16)
        tab_b1 = dram.tile([NB, D], bf16)
        tab_s1 = dram.tile([NS, D], bf16)

        bn_wb = dram.tile([WROWS, D], bf16)
        wb_full = dram.tile([8 * WROWS, D], bf16)
        nc.sync.dma_start(bn_xb[:], d_x[0:NLB, :])
        nc.sync.dma_start(bn_xs[:], d_x[NLB:NLB + NLS, :])
        nc.sync.dma_start(bn_wb[:], d_wbsh[:])
        nc.gpsimd.collective_compute(
            "AllGather", mybir.AluOpType.bypass,
            replica_groups=[list(range(NCORES))],
            ins=[bn_wb[:].opt()], outs=[wb_full[:].opt()])
        nc.gpsimd.collective_compute(
            "AllGather", mybir.AluOpType.bypass,
            replica_groups=[list(range(NCORES))],
            ins=[bn_xb[:].opt()], outs=[tab_b0[:].opt()])
        nc.gpsimd.collective_compute(
            "AllGather", mybir.AluOpType.bypass,
            replica_groups=[list(range(NCORES))],
            ins=[bn_xs[:].opt()], outs=[tab_s0[:].opt()])

        # ---- weights (each core uploaded partitions 16c..16c+16; the
        # AllGather of the row blocks reassembles the full [128, NWB])
        t_wb = wpool.tile([P, NWB], bf16, tag="wb")
        nc.sync.dma_start(
            t_wb[:],
            wb_full[:].rearrange("(p k) b -> p (k b)", p=P))
        off = 0
        wname = ["Wl1bb", "Wl1sb", "Wr1b", "Wl1bs", "Wr1s",
                 "Wl2bb", "Wl2sb", "Wr2b"]
        wmat = {}
        for n in wname:
            wmat[n] = t_wb[:, off:off + D]; off += D
        w_WhT = t_wb[:, off:off + 8]; off += 8
        w_ones = t_wb[:, off:off + S]; off += S    # all-ones (row 0 used)
        w_ident = t_wb[:, off:off + D]; off += D
        t_wf = wpool.tile([P, 4], f32, tag="wf")
        nc.sync.dma_start(t_wf[:], d_aux[:, 0:16].bitcast(f32))
        w_b1b = t_wf[:, 0:1]
        w_b1s = t_wf[:, 1:2]
        w_b2b = t_wf[:, 2:3]
        w_bh = t_wf[:, 3:4]

        # ---- persistent idx / rel (int8) / iota in SBUF (both layers)
        i8 = mybir.dt.int8
        t_iota = ipool.tile([P, S], i8, tag="iota")
        nc.sync.dma_start(
            t_iota[:],
            d_aux[:, aux_off["iota"]:aux_off["iota"] + S].bitcast(i8))
        t_idx = {}
        t_rel = {}
        for name, t in types.items():
            t_idx[name] = []
            for b, cb in enumerate(t["bcols"]):
                c0, w = idx_off[(name, b)]
                ti = ipool.tile([P, w], i16, tag=f"idx_{name}_{b}")
                # replicate [16, cols] across partitions by doubling
                nc.sync.dma_start(ti[0:16, :], d_idx_all[:, c0:c0 + w])
                for r in [16, 32, 64]:
                    nc.sync.dma_start(ti[r:2 * r, :], ti[0:r, :])
                t_idx[name].append(ti)
            W = t["Wtot"]
            a = aux_off[f"rel_{name}"]
            tr = ipool.tile([P, W], i8, tag=f"rel_{name}")
            nc.sync.dma_start(tr[:], d_aux[:, a:a + W].bitcast(i8))
            t_rel[name] = tr

        # ---- materialize per-dst scale rows (1/cnt broadcast to 128
        # partitions) via K=1 outer-product matmuls: ones[1,128] x ivc[1,n]
        t_scale = {}
        for name, nloc, vbase in [("bb", NLB, 0), ("sb", NLS, NLBP),
                                  ("bs", NLS, NLBP + NLSP)]:
            sc = kpool.tile([P, nloc], bf16, tag=f"sc_{name}")
            for g in range((nloc + GROUP - 1) // GROUP):
                ncg = min(GROUP, nloc - g * GROUP)
                t_st = spool.tile([1, GROUP], bf16, tag="ivst")
                nc.sync.dma_start(
                    t_st[:, :ncg],
                    d_ivc[:, vbase + g * GROUP:vbase + g * GROUP + ncg])
                ps_sc = s2pool.tile([P, GROUP], f32, space="PSUM", tag="s2")
                nc.tensor.matmul(ps_sc[:, :ncg], lhsT=w_ones[0:1, :],
                                 rhs=t_st[0:1, :ncg], start=True, stop=True)
                nc.vector.tensor_copy(out=sc[:, g * GROUP:g * GROUP + ncg],
                                      in_=ps_sc[:, :ncg])
            t_scale[name] = sc

        # L1 b-output kept resident as the L2 dense rhs (x1^T)
        t_keep = kpool.tile([P, NLB], bf16, tag="keep")

        def aggregate(tname, g, wbase, tabs):
            """Aggregate one group of `tname` from DRAM tables `tabs`
            (list per bucket of (tile, row_offset)) into a PSUM tile.
            Scatter-SUM via 0/1 selection; the 1/cnt scaling is applied
            per dst column from the materialized scale tile."""
            t = types[tname]
            wins = t["groups"][g]        # (bucket, k_local, col_off, span)
            meta = t["gb_meta"][g]       # bucket -> (slot_base, Nk)
            Wg = len(wins)
            ncols = max(c + s for (_, _, c, s) in wins)
            tr = t_rel[tname]
            gtiles = {}
            for b, (sbase, Nk) in sorted(meta.items()):
                t_gb = gpool.tile([P, (Nk // P) * D], bf16, tag="gb")
                tab_tile, roff = tabs[b]
                nc.gpsimd.dma_gather(
                    out_ap=t_gb[:].rearrange("p (k d) -> p k d", k=Nk // P),
                    in_ap=tab_tile[roff:roff + BUCK, :],
                    idxs_ap=t_idx[tname][b][:, sbase // 16:(sbase + Nk) // 16],
                    num_idxs=Nk, num_idxs_reg=Nk, elem_size=D,
                    single_packet=False)
                gtiles[b] = t_gb
            t_sel = selpool.tile([P, Wg * S], bf16, tag="sel")
            sel3 = t_sel[:].rearrange("p (w s) -> p w s", w=Wg)
            nc.vector.tensor_tensor(
                out=sel3,
                in0=tr[:, wbase:wbase + Wg, None].to_broadcast([P, Wg, S]),
                in1=t_iota[:, None, :].to_broadcast([P, Wg, S]),
                op=mybir.AluOpType.is_equal)
            t_ps = appool.tile([P, GROUP], f32, space="PSUM", tag="agg")
            for w, (b, k, coff, span) in enumerate(wins):
                nc.tensor.matmul(
                    t_ps[:, coff:coff + span],
                    lhsT=gtiles[b][:, k * D:(k + 1) * D],
                    rhs=t_sel[:, w * S:w * S + span],
                    start=(w == 0), stop=(w == Wg - 1))
            t_m = spool.tile([P, GROUP], bf16, tag="mT")
            nc.vector.tensor_copy(out=t_m[:, :ncols], in_=t_ps[:, :ncols])
            scol = g * GROUP
            nc.vector.tensor_tensor(
                out=t_m[:, :ncols], in0=t_m[:, :ncols],
                in1=t_scale[tname][:, scol:scol + ncols],
                op=mybir.AluOpType.mult)
            return t_m, ncols

        def transpose_out(src_tile, base, ncols, dst_dram, c0):
            """PE-transpose src_tile[:, base:base+ncols] bf16 into dst_dram
            rows c0..c0+ncols. One PSUM accumulation group for all chunks."""
            nch = (ncols + P - 1) // P
            # full 2KB zero region (1024 bf16 cols) so start=True owns a bank
            t_pt = tpool.tile([P, 2 * GROUP], bf16, space="PSUM", tag="tr")
            for k in range(nch):
                pk = min(P, ncols - k * P)
                nc.tensor.matmul(
                    t_pt[:pk, k * P:k * P + P],
                    lhsT=src_tile[:, base + k * P:base + k * P + pk],
                    rhs=w_ident[:], is_transpose=True,
                    start=(k == 0), stop=(k == nch - 1))
            t_tr = spool.tile([P, GROUP], bf16, tag="trs")
            if ncols == GROUP:   # full group: one copy + one rearranged DMA
                nc.vector.tensor_copy(out=t_tr[:], in_=t_pt[:, :GROUP])
                nc.sync.dma_start(
                    dst_dram[c0:c0 + GROUP, :].rearrange(
                        "(k p) f -> p k f", p=P),
                    t_tr[:].rearrange("p (k f) -> p k f", k=nch))
                return
            for k in range(nch):
                pk = min(P, ncols - k * P)
                nc.vector.tensor_copy(out=t_tr[:pk, k * P:k * P + P],
                                      in_=t_pt[:pk, k * P:k * P + P])
                nc.sync.dma_start(
                    dst_dram[c0 + k * P:c0 + k * P + pk, :],
                    t_tr[:pk, k * P:k * P + P])

        ngb = len(types["bb"]["groups"])
        ngs = len(types["bs"]["groups"])
        wb = {n: 0 for n in types}

        tabs_b0 = [(tab_b0, b * BUCK) for b in range(4)]
        tabs_s0 = [(tab_s0, b * BUCK) for b in range(2)]
        tabs_b1 = [(tab_b1, b * BUCK) for b in range(4)]
        tabs_s1 = [(tab_s1, b * BUCK) for b in range(2)]

        # stage-0 escape: touch output so the program is well-formed
        def dummy_out():
            t_d = spool.tile([8, GROUP], bf16, tag="yt")
            nc.vector.tensor_copy(out=t_d[:, :4], in_=t_wf[:8, :])
            nc.sync.dma_start(d_yT[:, 0:4], t_d[:, :4])

        if STAGE < 1:
            dummy_out()
            ngs = 0
        # ---- layer 1, s-dst groups first (frees ns1 AllGather early)
        for g in range(ngs):
            m_bs, ncols = aggregate("bs", g, wb["bs"], tabs_b0)
            wb["bs"] += len(types["bs"]["groups"][g])
            t_x = spool.tile([P, GROUP], bf16, tag="xg")
            nc.sync.dma_start(
                t_x[:, :ncols],
                bn_xs[g * GROUP:g * GROUP + ncols, :].rearrange("r f -> f r"))
            ps2 = s2pool.tile([P, GROUP], f32, space="PSUM", tag="s2")
            nc.tensor.matmul(ps2[:, :ncols], lhsT=wmat["Wl1bs"],
                             rhs=m_bs[:, :ncols], start=True, stop=False)
            nc.tensor.matmul(ps2[:, :ncols], lhsT=wmat["Wr1s"],
                             rhs=t_x[:, :ncols], start=False, stop=True)
            t_o = spool.tile([P, GROUP], bf16, tag="ob")
            nc.scalar.activation(out=t_o[:, :ncols], in_=ps2[:, :ncols],
                                 func=mybir.ActivationFunctionType.Lrelu,
                                 bias=w_b1s, alpha=0.01)
            transpose_out(t_o, 0, ncols, bn_ns1, g * GROUP)
        if STAGE >= 2:
            nc.gpsimd.collective_compute(
                "AllGather", mybir.AluOpType.bypass,
                replica_groups=[list(range(NCORES))],
                ins=[bn_ns1[:].opt()], outs=[tab_s1[:].opt()])
        if STAGE < 3:
            if STAGE >= 1:
                dummy_out()
            ngb = 0

        # ---- layer 1, b-dst groups
        ngsb = len(types["sb"]["groups"])   # sb dst locals < NS//NCORES
        for g in range(ngb):
            m_bb, ncols = aggregate("bb", g, wb["bb"], tabs_b0)
            wb["bb"] += len(types["bb"]["groups"][g])
            has_sb = g < ngsb
            if has_sb:
                m_sb, ncols_sb = aggregate("sb", g, wb["sb"], tabs_s0)
                wb["sb"] += len(types["sb"]["groups"][g])
            t_x = spool.tile([P, GROUP], bf16, tag="xg")
            nc.sync.dma_start(
                t_x[:, :ncols],
                bn_xb[g * GROUP:g * GROUP + ncols, :].rearrange("r f -> f r"))
            ps2 = s2pool.tile([P, GROUP], f32, space="PSUM", tag="s2")
            nc.tensor.matmul(ps2[:, :ncols], lhsT=wmat["Wl1bb"],
                             rhs=m_bb[:, :ncols], start=True, stop=False)
            if has_sb:
                nc.tensor.matmul(ps2[:, :ncols_sb], lhsT=wmat["Wl1sb"],
                                 rhs=m_sb[:, :ncols_sb], start=False, stop=False)
            nc.tensor.matmul(ps2[:, :ncols], lhsT=wmat["Wr1b"],
                             rhs=t_x[:, :ncols], start=False, stop=True)
            t_o1 = t_keep[:, g * GROUP:g * GROUP + ncols]
            nc.scalar.activation(out=t_o1, in_=ps2[:, :ncols],
                                 func=mybir.ActivationFunctionType.Lrelu,
                                 bias=w_b1b, alpha=0.01)
            transpose_out(t_keep, g * GROUP, ncols, bn_nb1, g * GROUP)
        if STAGE >= 3 and ngb:
            nc.gpsimd.collective_compute(
                "AllGather", mybir.AluOpType.bypass,
                replica_groups=[list(range(NCORES))],
                ins=[bn_nb1[:].opt()], outs=[tab_b1[:].opt()])
        if STAGE == 5:
            nc.sync.dma_start(d_dbg_b[:], bn_nb1[:])
            nc.sync.dma_start(d_dbg_s[:], bn_ns1[:])
        if STAGE < 4 or STAGE == 5:
            ngb = 0

        # ---- layer 2 (b-dst only) + heads
        wb["bb"] = 0
        wb["sb"] = 0
        for g in range(ngb):
            m_bb, ncols = aggregate("bb", g, wb["bb"], tabs_b1)
            wb["bb"] += len(types["bb"]["groups"][g])
            has_sb = g < ngsb
            if has_sb:
                m_sb, ncols_sb = aggregate("sb", g, wb["sb"], tabs_s1)
                wb["sb"] += len(types["sb"]["groups"][g])
            ps2 = s2pool.tile([P, GROUP], f32, space="PSUM", tag="s2")
            nc.tensor.matmul(ps2[:, :ncols], lhsT=wmat["Wl2bb"],
                             rhs=m_bb[:, :ncols], start=True, stop=False)
            if has_sb:
                nc.tensor.matmul(ps2[:, :ncols_sb], lhsT=wmat["Wl2sb"],
                                 rhs=m_sb[:, :ncols_sb], start=False, stop=False)
            nc.tensor.matmul(ps2[:, :ncols], lhsT=wmat["Wr2b"],
                             rhs=t_keep[:, g * GROUP:g * GROUP + ncols],
                             start=False, stop=True)
            t_o = spool.tile([P, GROUP], bf16, tag="ob")
            nc.scalar.activation(out=t_o[:, :ncols], in_=ps2[:, :ncols],
                                 func=mybir.ActivationFunctionType.Lrelu,
                                 bias=w_b2b, alpha=0.01)
            ps3 = tpool.tile([8, GROUP], f32, space="PSUM", tag="hd")
            nc.tensor.matmul(ps3[:, :ncols], lhsT=w_WhT,
                             rhs=t_o[:, :ncols], start=True, stop=True)
            t_y = spool.tile([8, GROUP], bf16, tag="yt")
            nc.vector.tensor_copy(out=t_y[:, :ncols], in_=ps3[:, :ncols])
            nc.sync.dma_start(d_yT[:, g * GROUP:g * GROUP + ncols],
                              t_y[:, :ncols])

    nc.compile()
    return nc


LAST_HW_NS = None
LAST_EXEC_S = None
LAST_PREP_S = None


def kernel(x_b, x_s, Wl, bl, Wr, Wh, bh, ei_bb, ei_sb, ei_bs):
    t_prep0 = time.time()
    x_b = np.asarray(x_b, np.float32); x_s = np.asarray(x_s, np.float32)
    Wl = np.asarray(Wl, np.float32); bl = np.asarray(bl, np.float32)
    Wr = np.asarray(Wr, np.float32); Wh = np.asarray(Wh, np.float32)
    bh = np.asarray(bh, np.float32)
    ei_bb = np.asarray(ei_bb); ei_sb = np.asarray(ei_sb); ei_bs = np.asarray(ei_bs)

    # warm the PJRT/axon connection and the jit/compile/dispatch machinery
    # outside the timed launch (one trivial 8-core program)
    import jax
    jax.device_put(np.zeros(16, np.float32), jax.devices()[0]).block_until_ready()
    nc_w = bacc.Bacc("TRN2", target_bir_lowering=False, debug=False,
                     num_devices=NCORES)
    dw_i = nc_w.dram_tensor("wi", [1, 16], mybir.dt.float32,
                            kind="ExternalInput")
    dw_o = nc_w.dram_tensor("wo", [1, 16], mybir.dt.float32,
                            kind="ExternalOutput")
    with tile.TileContext(nc_w):
        nc_w.sync.dma_start(dw_o[:], dw_i[:])
    nc_w.compile()
    try:
        run_bass_kernel_spmd(nc_w, [{"wi": np.zeros((1, 16), np.float32)}] * NCORES,
                             core_ids=list(range(NCORES)))
    except Exception:
        pass   # transient device wedge: the real launch below may still work

    def tr_b(v):
        return (v % NCORES) * NLB + v // NCORES

    def tr_s(v):
        return (v % NCORES) * NLS + v // NCORES

    # One packing shared by both layers: src ids translated to block order.
    pc_bb = _shard_edges(tr_b(ei_bb[0]), ei_bb[1], NB)
    pc_sb = _shard_edges(tr_s(ei_sb[0]), ei_sb[1], NB)   # dst b-nodes < NS
    pc_bs = _shard_edges(tr_b(ei_bs[0]), ei_bs[1], NS)
    i_bb, r_bb, v_bb, g_bb, m_bb = _pack_type(pc_bb, NLB, NB)
    i_sb, r_sb, v_sb, g_sb, m_sb = _pack_type(pc_sb, NS // NCORES, NS)
    i_bs, r_bs, v_bs, g_bs, m_bs = _pack_type(pc_bs, NLS, NB)

    cfg = {"types": {
        "bb": {"tab": "b", "Wtot": r_bb.shape[2], "groups": g_bb,
               "gb_meta": m_bb, "bcols": [a.shape[2] for a in i_bb]},
        "sb": {"tab": "s", "Wtot": r_sb.shape[2], "groups": g_sb,
               "gb_meta": m_sb, "bcols": [a.shape[2] for a in i_sb]},
        "bs": {"tab": "b", "Wtot": r_bs.shape[2], "groups": g_bs,
               "gb_meta": m_bs, "bcols": [a.shape[2] for a in i_bs]},
    }}
    nc = _build(cfg)

    # ---- pack weights
    NWB = 8 * D + 8 + S + D
    NWB += (-NWB) % 128
    wbf = np.zeros((P, NWB), BF16)
    off = 0
    for M in [Wl[0, 0], Wl[0, 1], Wr[0, 0] + Wr[0, 1], Wl[0, 2], Wr[0, 2],
              Wl[1, 0], Wl[1, 1], Wr[1, 0] + Wr[1, 1]]:
        wbf[:, off:off + D] = M.astype(BF16); off += D
    wbf[:, off:off + 8] = Wh.T.astype(BF16); off += 8
    wbf[:, off:off + S] = np.ones((1, S), BF16); off += S
    wbf[:, off:off + D] = np.eye(D, dtype=BF16); off += D
    wff = np.zeros((P, 4), np.float32)
    wff[:, 0] = bl[0, 0] + bl[0, 1]
    wff[:, 1] = bl[0, 2]
    wff[:, 2] = bl[1, 0] + bl[1, 1]
    wff[:8, 3] = bh

    xb16 = x_b.astype(BF16)
    xs16 = x_s.astype(BF16)

    # mirror _build's blob layout exactly
    NLBP = NLB + (-NLB) % GROUP
    NLSP = NLS + (-NLS) % GROUP
    iota8 = np.tile(np.arange(S, dtype=np.int8)[None, :], (P, 1))
    in_maps = []
    for c in range(NCORES):
        idx_secs = []
        for arrs in [i_bb, i_sb, i_bs]:
            for a in arrs:
                sec = a[c] if a.shape[2] >= 16 else np.zeros((16, 16), np.int16)
                idx_secs.append(sec)
        idx_all = np.concatenate(idx_secs, axis=1)
        pad = (-idx_all.shape[1]) % 128
        if pad:
            idx_all = np.concatenate(
                [idx_all, np.zeros((16, pad), np.int16)], 1)
        ivc = np.zeros(NLBP + 2 * NLSP, BF16)
        ivc[0:NLB] = v_bb[c]
        ivc[NLBP:NLBP + NLS] = v_sb[c]
        ivc[NLBP + NLSP:NLBP + NLSP + NLS] = v_bs[c]
        aux_secs = [wff.view(np.uint8)]
        ab = 16
        for r in [r_bb, r_sb, r_bs]:
            aux_secs.append(r[c].view(np.uint8))
            ab += r.shape[2]
            pad = (-ab) % 4
            if pad:
                aux_secs.append(np.zeros((P, pad), np.uint8))
                ab += pad
        aux_secs.append(iota8.view(np.uint8))
        ab += S
        pad = (-ab) % 256
        if pad:
            aux_secs.append(np.zeros((P, pad), np.uint8))
            ab += pad
        aux = np.ascontiguousarray(np.concatenate(aux_secs, axis=1))
        aux_rows = aux.reshape(-1).view(BF16).reshape(-1, D)
        idx_rows = (np.ascontiguousarray(idx_all).reshape(-1)
                    .view(BF16).reshape(-1, D))
        wb_rows = wbf[16 * c:16 * (c + 1)].reshape(-1, D)
        in_maps.append({
            "blob": np.concatenate(
                [xb16[c::NCORES], xs16[c::NCORES], ivc.reshape(-1, D),
                 wb_rows, aux_rows, idx_rows], 0),
        })
    global LAST_HW_NS, LAST_EXEC_S, LAST_PREP_S
    LAST_PREP_S = time.time() - t_prep0

    _t0 = time.time()
    res = None
    for _attempt in range(3):
        try:
            res = run_bass_kernel_spmd(nc, in_maps,
                                       core_ids=list(range(NCORES)))
            break
        except Exception:
            # transient NRT_EXEC_UNIT_UNRECOVERABLE wedge: retrying the
            # same launch usually succeeds
            if _attempt == 2:
                raise
            time.sleep(2.0)
    _exec = time.time() - _t0
    if res.exec_time_ns:
        LAST_HW_NS = int(res.exec_time_ns)
    LAST_EXEC_S = (_exec,)
    global LAST_DBG
    if "dbg_b" in res.results[0]:
        LAST_DBG = ([res.results[c]["dbg_b"] for c in range(NCORES)],
                    [res.results[c]["dbg_s"] for c in range(NCORES)])

    y = np.empty((NB, 8), np.float32)
    for c in range(NCORES):
        y[np.arange(NLB) * NCORES + c] = res.results[c]["yT"].T.astype(np.float32)
    y += bh[None, :]
    return y


# revision 13
# speedup vs baseline: 1.0123x; 1.0123x over previous
"""HGNN (2-layer hetero GraphSAGE + 8 heads) on 8 trn2 NeuronCores.

Single-launch design. Nodes are sharded dst-interleaved (core = v % 8,
local = v // 8); all src gather indices are pre-translated into the
"concatenated core blocks" order pos(v) = (v%8)*n_loc + v//8, which is the
layout produced by AllGather of per-core blocks. Both layers share one edge
packing (same graph), and the inter-layer halo exchange runs ON DEVICE:

  AllGather(x shards) -> tab0 -> layer1 -> PE-transpose -> AllGather -> tab1
  -> layer2 -> 8-head matmul -> yT

Features, weights and selection matrices are bf16 (fp32 PSUM accumulation);
the head bias is added host-side in fp32. All per-core inputs ship as ONE
consolidated tensor (~5.7MB/core vs ~170MB/core replicated in the two-launch
version): each named input costs ~60ms of PJRT-over-axon latency on top of
~21-50MB/s bandwidth, and the weight block itself is sharded 16 rows per
core and AllGathered on device.

Aggregation per 512-dst PSUM group: edges (dst-sorted) are cut into 128-edge
windows on a column grid uniform across cores, bucketed by src block of
25000 rows (int16 gather indices, 8x partition-replicated on device). Per
window one indirect DMA gathers 128 src rows; the 0/1 selection
sel[e, j] = (rel_dst[e] == j) is one int8 DVE is_equal; PE accumulates
g.T @ sel into PSUM, yielding the scatter-SUM s^T in [feat, dst]
orientation; the 1/cnt mean scaling is applied per dst column from scale
rows materialized on device by K=1 ones-outer-product matmuls.
"""
import time
import numpy as np
import ml_dtypes

import concourse.bass as bass
import concourse.bacc as bacc
import concourse.mybir as mybir
import concourse.tile as tile
from concourse.bass_utils import run_bass_kernel_spmd

P = 128
D = 128
NCORES = 8
GROUP = 512       # psum columns per accumulation group
S = 128           # max dst-column span per 128-edge window
BUCK = 25000      # src rows per int16 gather bucket
NB, NS = 100000, 50000
NLB, NLS = NB // NCORES, NS // NCORES   # 12500, 6250
BF16 = ml_dtypes.bfloat16


# ---------------------------------------------------------------- host prep
def _shard_edges(src, dst, n_dst):
    """Split edges by dst core; per core return (src, dst_local) dst-sorted.
    src must already be translated to concatenated-block order."""
    core = dst % NCORES
    loc = dst // NCORES
    out = []
    for c in range(NCORES):
        m = core == c
        s, d = src[m], loc[m]
        o = np.argsort(d, kind="stable")
        out.append((s[o].astype(np.int64), d[o].astype(np.int64)))
    return out


def _pack_type(per_core, n_loc, n_src):
    """Bucketed uniform-across-cores window packing for dma_gather.

    Returns (idx16 per bucket: list of [NCORES, 128, cols_b],
             rel [NCORES, P, Wtot] bf16, invc [NCORES, P, Wtot] bf16,
             groups: per group list of (bucket, k_local, col_off, span),
             gb_meta: per group dict bucket -> (idx_col_base, Nk))."""
    nbuck = (n_src + BUCK - 1) // BUCK
    ngroups = (n_loc + GROUP - 1) // GROUP
    pcb = [[None] * nbuck for _ in range(NCORES)]
    cumb = [[None] * nbuck for _ in range(NCORES)]
    counts_all = []
    for cc, (s, d) in enumerate(per_core):
        counts_all.append(np.bincount(d, minlength=n_loc))
        for b in range(nbuck):
            m = (s >= b * BUCK) & (s < (b + 1) * BUCK)
            sb_, db_ = s[m], d[m]
            pcb[cc][b] = (sb_ - b * BUCK, db_)
            cnt = np.bincount(db_, minlength=n_loc)
            cumb[cc][b] = np.concatenate([[0], np.cumsum(cnt)])
    invc_dst = [1.0 / np.maximum(c, 1) for c in counts_all]

    groups, gb_meta = [], []
    rel_cols = [[] for _ in range(NCORES)]
    idx_flat = [[[] for _ in range(nbuck)] for _ in range(NCORES)]
    idx_base = [0] * nbuck
    for g in range(ngroups):
        c0, c1 = g * GROUP, min((g + 1) * GROUP, n_loc)
        wins, meta = [], {}
        for b in range(nbuck):
            k_local = 0
            c = c0
            while c < c1:
                span = min(S, c1 - c)
                while span > 1:
                    ok = all(cumb[cc][b][c + span] - cumb[cc][b][c] <= P
                             for cc in range(NCORES))
                    if ok:
                        break
                    span -= 1
                for cc in range(NCORES):
                    s_arr, d_arr = pcb[cc][b]
                    a2, b2 = cumb[cc][b][c], cumb[cc][b][c + span]
                    n = b2 - a2
                    assert n <= P
                    icol = np.zeros(P, np.int16)
                    rcol = np.full(P, -1, np.int8)
                    icol[:n] = s_arr[a2:b2].astype(np.int16)
                    rcol[:n] = (d_arr[a2:b2] - c).astype(np.int8)
                    idx_flat[cc][b].append(icol)
                    rel_cols[cc].append(rcol)
                wins.append((b, k_local, c - c0, span))
                k_local += 1
                c += span
            if k_local:
                meta[b] = (idx_base[b], k_local * P)
                idx_base[b] += k_local * P
        groups.append(wins)
        gb_meta.append(meta)

    # int16 device layout per bucket: flat i at [i%16, i//16]; the 8x
    # partition replication dma_gather wants is done on device.
    idx16 = []
    for b in range(nbuck):
        per_core_arr = []
        for cc in range(NCORES):
            flat = (np.concatenate(idx_flat[cc][b]) if idx_flat[cc][b]
                    else np.zeros(16, np.int16))
            per_core_arr.append(flat.reshape(-1, 16).T)   # [16, cols]
        idx16.append(np.stack(per_core_arr).astype(np.int16))
    rel = np.stack([np.stack(cols, 1) for cols in rel_cols]).astype(np.int8)
    ivcd = np.stack(invc_dst).astype(BF16)    # [NCORES, n_loc] per-dst 1/cnt
    return idx16, rel, ivcd, groups, gb_meta


# ------------------------------------------------------------- device build
def _build(cfg):
    """Build the merged 2-layer SPMD program. cfg keys:
      types: dict name -> dict(tab ('b'|'s'), Wtot, groups, gb_meta, bcols)
      stage: 0=allgather only, 1=+L1 s-groups, 2=+ns1 allgather,
             3=+L1 b-groups + nb1 allgather, 4=full
    """
    STAGE = 4
    t_build0 = time.time()
    nc = bacc.Bacc("TRN2", target_bir_lowering=False, debug=False,
                   num_devices=NCORES)
    f32, bf16, i16 = mybir.dt.float32, mybir.dt.bfloat16, mybir.dt.int16
    types = cfg["types"]

    # ONE consolidated input tensor (each named input costs ~60ms of
    # PJRT-over-axon transfer latency). Layout, in 256B rows of [R, D] bf16:
    #   rows 0..NLB+NLS      : xb shard rows then xs shard rows
    #   then ivc [1, IVC] bf16: per-dst 1/cnt for bb | sb | bs (512-padded)
    #   then wbsh [16, NWB] bf16: this core's 16 partitions of the weights
    #        (the full [128, NWB] weight block is AllGathered on device)
    #   then aux [128, XB] u8: wts_f | rel per type | int8 iota
    #   then idx [16, IC] i16: per (type, bucket) column sections (IC % 128 == 0)
    NWB = 8 * D + 8 + S + D    # 8 mats | WhT | ones row | identity
    NWB += (-NWB) % 128        # 1408: 16-row shard must be whole 256B rows
    NLBP = NLB + (-NLB) % GROUP      # 12800
    NLSP = NLS + (-NLS) % GROUP      # 6400
    IVC = NLBP + 2 * NLSP            # bb | sb | bs scale vectors
    idx_off = {}
    col = 0
    for name, t in types.items():
        for b, cb in enumerate(t["bcols"]):
            w = max(cb, 16)
            idx_off[(name, b)] = (col, w)
            col += w
    col += (-col) % 128                           # 16*col*2 % 256 == 0
    IC = col
    aux_off = {}
    ab = 0
    aux_off["wf"] = ab; ab += 16                  # [128, 4] f32
    for name, t in types.items():
        aux_off[f"rel_{name}"] = ab; ab += t["Wtot"]   # int8
        ab += (-ab) % 4
    aux_off["iota"] = ab; ab += S                 # [128, S] int8
    ab += (-ab) % 256
    XB = ab
    XROWS = NLB + NLS
    VROWS = IVC * 2 // 256
    WROWS = 16 * NWB * 2 // 256      # = NWB // 8; NWB % 8 == 0
    AROWS = P * XB // 256
    IROWS = 16 * IC * 2 // 256
    d_all = nc.dram_tensor(
        "blob", [XROWS + VROWS + WROWS + AROWS + IROWS, D], bf16,
        kind="ExternalInput")
    d_x = d_all   # rows 0..XROWS
    d_ivc = (d_all[XROWS:XROWS + VROWS, :]
             .rearrange("(o k) b -> o (k b)", o=1))
    r0 = XROWS + VROWS
    d_wbsh = d_all[r0:r0 + WROWS, :]
    r0 += WROWS
    d_aux = (d_all[r0:r0 + AROWS, :].bitcast(mybir.dt.uint8)
             .rearrange("(p k) b -> p (k b)", p=P))
    d_idx_all = (d_all[r0 + AROWS:r0 + AROWS + IROWS, :]
                 .bitcast(mybir.dt.int16)
                 .rearrange("(p k) b -> p (k b)", p=16))
    d_yT = nc.dram_tensor("yT", [8, NLB], bf16, kind="ExternalOutput")
    d_dbg_b = d_dbg_s = None
    if STAGE == 5:   # debug: emit layer-1 outputs (block rows, bf16)
        d_dbg_b = nc.dram_tensor("dbg_b", [NLB, D], bf16, kind="ExternalOutput")
        d_dbg_s = nc.dram_tensor("dbg_s", [NLS, D], bf16, kind="ExternalOutput")

    from contextlib import ExitStack
    with tile.TileContext(nc) as tc, ExitStack() as ctx:
        dram = ctx.enter_context(tc.tile_pool(name="dram", bufs=1, space="DRAM"))
        wpool = ctx.enter_context(tc.tile_pool(name="w", bufs=1))
        ipool = ctx.enter_context(tc.tile_pool(name="i", bufs=1))
        kpool = ctx.enter_context(tc.tile_pool(name="k", bufs=1))
        gpool = ctx.enter_context(tc.tile_pool(name="g", bufs=4))
        selpool = ctx.enter_context(tc.tile_pool(name="sel", bufs=2))
        spool = ctx.enter_context(tc.tile_pool(name="s", bufs=3))
        appool = ctx.enter_context(tc.tile_pool(name="ap", bufs=2, space="PSUM"))
        s2pool = ctx.enter_context(tc.tile_pool(name="s2", bufs=2, space="PSUM"))
        tpool = ctx.enter_context(tc.tile_pool(name="t", bufs=1, space="PSUM"))

        # ---- DRAM scratch: bounce blocks + gathered tables
        bn_xb = dram.tile([NLB, D], bf16)
        bn_xs = dram.tile([NLS, D], bf16)
        tab_b0 = dram.tile([NB, D], bf16)
        tab_s0 = dram.tile([NS, D], bf16)
        bn_nb1 = dram.tile([NLB, D], bf16)
        bn_ns1 = dram.tile([NLS, D], bf16)
        tab_b1 = dram.tile([NB, D], bf16)
        tab_s1 = dram.tile([NS, D], bf16)

        bn_wb = dram.tile([WROWS, D], bf16)
        wb_full = dram.tile([8 * WROWS, D], bf16)
        nc.sync.dma_start(bn_xb[:], d_x[0:NLB, :])
        nc.sync.dma_start(bn_xs[:], d_x[NLB:NLB + NLS, :])
        nc.sync.dma_start(bn_wb[:], d_wbsh[:])
        nc.gpsimd.collective_compute(
            "AllGather", mybir.AluOpType.bypass,
            replica_groups=[list(range(NCORES))],
            ins=[bn_wb[:].opt()], outs=[wb_full[:].opt()])
        nc.gpsimd.collective_compute(
            "AllGather", mybir.AluOpType.bypass,
            replica_groups=[list(range(NCORES))],
            ins=[bn_xb[:].opt()], outs=[tab_b0[:].opt()])
        nc.gpsimd.collective_compute(
            "AllGather", mybir.AluOpType.bypass,
            replica_groups=[list(range(NCORES))],
            ins=[bn_xs[:].opt()], outs=[tab_s0[:].opt()])

        # ---- weights (each core uploaded partitions 16c..16c+16; the
        # AllGather of the row blocks reassembles the full [128, NWB])
        t_wb = wpool.tile([P, NWB], bf16, tag="wb")
        nc.sync.dma_start(
            t_wb[:],
            wb_full[:].rearrange("(p k) b -> p (k b)", p=P))
        off = 0
        wname = ["Wl1bb", "Wl1sb", "Wr1b", "Wl1bs", "Wr1s",
                 "Wl2bb", "Wl2sb", "Wr2b"]
        wmat = {}
        for n in wname:
            wmat[n] = t_wb[:, off:off + D]; off += D
        w_WhT = t_wb[:, off:off + 8]; off += 8
        w_ones = t_wb[:, off:off + S]; off += S    # all-ones (row 0 used)
        w_ident = t_wb[:, off:off + D]; off += D
        t_wf = wpool.tile([P, 4], f32, tag="wf")
        nc.sync.dma_start(t_wf[:], d_aux[:, 0:16].bitcast(f32))
        w_b1b = t_wf[:, 0:1]
        w_b1s = t_wf[:, 1:2]
        w_b2b = t_wf[:, 2:3]
        w_bh = t_wf[:, 3:4]

        # ---- persistent idx / rel (int8) / iota in SBUF (both layers)
        i8 = mybir.dt.int8
        t_iota = ipool.tile([P, S], i8, tag="iota")
        nc.sync.dma_start(
            t_iota[:],
            d_aux[:, aux_off["iota"]:aux_off["iota"] + S].bitcast(i8))
        t_idx = {}
        t_rel = {}
        for name, t in types.items():
            t_idx[name] = []
            for b, cb in enumerate(t["bcols"]):
                c0, w = idx_off[(name, b)]
                ti = ipool.tile([P, w], i16, tag=f"idx_{name}_{b}")
                # replicate [16, cols] across partitions by doubling
                nc.sync.dma_start(ti[0:16, :], d_idx_all[:, c0:c0 + w])
                for r in [16, 32, 64]:
                    nc.sync.dma_start(ti[r:2 * r, :], ti[0:r, :])
                t_idx[name].append(ti)
            W = t["Wtot"]
            a = aux_off[f"rel_{name}"]
            tr = ipool.tile([P, W], i8, tag=f"rel_{name}")
            nc.sync.dma_start(tr[:], d_aux[:, a:a + W].bitcast(i8))
            t_rel[name] = tr

        # ---- materialize per-dst scale rows (1/cnt broadcast to 128
        # partitions) via K=1 outer-product matmuls: ones[1,128] x ivc[1,n]
        t_scale = {}
        for name, nloc, vbase in [("bb", NLB, 0), ("sb", NLS, NLBP),
                                  ("bs", NLS, NLBP + NLSP)]:
            sc = kpool.tile([P, nloc], bf16, tag=f"sc_{name}")
            for g in range((nloc + GROUP - 1) // GROUP):
                ncg = min(GROUP, nloc - g * GROUP)
                t_st = spool.tile([1, GROUP], bf16, tag="ivst")
                nc.sync.dma_start(
                    t_st[:, :ncg],
                    d_ivc[:, vbase + g * GROUP:vbase + g * GROUP + ncg])
                ps_sc = s2pool.tile([P, GROUP], f32, space="PSUM", tag="s2")
                nc.tensor.matmul(ps_sc[:, :ncg], lhsT=w_ones[0:1, :],
                                 rhs=t_st[0:1, :ncg], start=True, stop=True)
                nc.vector.tensor_copy(out=sc[:, g * GROUP:g * GROUP + ncg],
                                      in_=ps_sc[:, :ncg])
            t_scale[name] = sc

        # L1 b-output kept resident as the L2 dense rhs (x1^T)
        t_keep = kpool.tile([P, NLB], bf16, tag="keep")

        def aggregate(tname, g, wbase, tabs):
            """Aggregate one group of `tname` from DRAM tables `tabs`
            (list per bucket of (tile, row_offset)) into a PSUM tile.
            Scatter-SUM via 0/1 selection; the 1/cnt scaling is applied
            per dst column from the materialized scale tile."""
            t = types[tname]
            wins = t["groups"][g]        # (bucket, k_local, col_off, span)
            meta = t["gb_meta"][g]       # bucket -> (slot_base, Nk)
            Wg = len(wins)
            ncols = max(c + s for (_, _, c, s) in wins)
            tr = t_rel[tname]
            gtiles = {}
            for b, (sbase, Nk) in sorted(meta.items()):
                t_gb = gpool.tile([P, (Nk // P) * D], bf16, tag="gb")
                tab_tile, roff = tabs[b]
                nc.gpsimd.dma_gather(
                    out_ap=t_gb[:].rearrange("p (k d) -> p k d", k=Nk // P),
                    in_ap=tab_tile[roff:roff + BUCK, :],
                    idxs_ap=t_idx[tname][b][:, sbase // 16:(sbase + Nk) // 16],
                    num_idxs=Nk, num_idxs_reg=Nk, elem_size=D,
                    single_packet=False)
                gtiles[b] = t_gb
            t_sel = selpool.tile([P, Wg * S], bf16, tag="sel")
            sel3 = t_sel[:].rearrange("p (w s) -> p w s", w=Wg)
            nc.vector.tensor_tensor(
                out=sel3,
                in0=tr[:, wbase:wbase + Wg, None].to_broadcast([P, Wg, S]),
                in1=t_iota[:, None, :].to_broadcast([P, Wg, S]),
                op=mybir.AluOpType.is_equal)
            t_ps = appool.tile([P, GROUP], f32, space="PSUM", tag="agg")
            for w, (b, k, coff, span) in enumerate(wins):
                nc.tensor.matmul(
                    t_ps[:, coff:coff + span],
                    lhsT=gtiles[b][:, k * D:(k + 1) * D],
                    rhs=t_sel[:, w * S:w * S + span],
                    start=(w == 0), stop=(w == Wg - 1))
            t_m = spool.tile([P, GROUP], bf16, tag="mT")
            nc.vector.tensor_copy(out=t_m[:, :ncols], in_=t_ps[:, :ncols])
            scol = g * GROUP
            nc.vector.tensor_tensor(
                out=t_m[:, :ncols], in0=t_m[:, :ncols],
                in1=t_scale[tname][:, scol:scol + ncols],
                op=mybir.AluOpType.mult)
            return t_m, ncols

        def transpose_out(src_tile, base, ncols, dst_dram, c0):
            """PE-transpose src_tile[:, base:base+ncols] bf16 into dst_dram
            rows c0..c0+ncols. One PSUM accumulation group for all chunks."""
            nch = (ncols + P - 1) // P
            # full 2KB zero region (1024 bf16 cols) so start=True owns a bank
            t_pt = tpool.tile([P, 2 * GROUP], bf16, space="PSUM", tag="tr")
            for k in range(nch):
                pk = min(P, ncols - k * P)
                nc.tensor.matmul(
                    t_pt[:pk, k * P:k * P + P],
                    lhsT=src_tile[:, base + k * P:base + k * P + pk],
                    rhs=w_ident[:], is_transpose=True,
                    start=(k == 0), stop=(k == nch - 1))
            t_tr = spool.tile([P, GROUP], bf16, tag="trs")
            if ncols == GROUP:   # full group: one copy + one rearranged DMA
                nc.vector.tensor_copy(out=t_tr[:], in_=t_pt[:, :GROUP])
                nc.sync.dma_start(
                    dst_dram[c0:c0 + GROUP, :].rearrange(
                        "(k p) f -> p k f", p=P),
                    t_tr[:].rearrange("p (k f) -> p k f", k=nch))
                return
            for k in range(nch):
                pk = min(P, ncols - k * P)
                nc.vector.tensor_copy(out=t_tr[:pk, k * P:k * P + P],
                                      in_=t_pt[:pk, k * P:k * P + P])
                nc.sync.dma_start(
                    dst_dram[c0 + k * P:c0 + k * P + pk, :],
                    t_tr[:pk, k * P:k * P + P])

        ngb = len(types["bb"]["groups"])
        ngs = len(types["bs"]["groups"])
        wb = {n: 0 for n in types}

        tabs_b0 = [(tab_b0, b * BUCK) for b in range(4)]
        tabs_s0 = [(tab_s0, b * BUCK) for b in range(2)]
        tabs_b1 = [(tab_b1, b * BUCK) for b in range(4)]
        tabs_s1 = [(tab_s1, b * BUCK) for b in range(2)]

        # stage-0 escape: touch output so the program is well-formed
        def dummy_out():
            t_d = spool.tile([8, GROUP], bf16, tag="yt")
            nc.vector.tensor_copy(out=t_d[:, :4], in_=t_wf[:8, :])
            nc.sync.dma_start(d_yT[:, 0:4], t_d[:, :4])

        if STAGE < 1:
            dummy_out()
            ngs = 0
        # ---- layer 1, s-dst groups first (frees ns1 AllGather early)
        for g in range(ngs):
            m_bs, ncols = aggregate("bs", g, wb["bs"], tabs_b0)
            wb["bs"] += len(types["bs"]["groups"][g])
            t_x = spool.tile([P, GROUP], bf16, tag="xg")
            nc.sync.dma_start(
                t_x[:, :ncols],
                bn_xs[g * GROUP:g * GROUP + ncols, :].rearrange("r f -> f r"))
            ps2 = s2pool.tile([P, GROUP], f32, space="PSUM", tag="s2")
            nc.tensor.matmul(ps2[:, :ncols], lhsT=wmat["Wl1bs"],
                             rhs=m_bs[:, :ncols], start=True, stop=False)
            nc.tensor.matmul(ps2[:, :ncols], lhsT=wmat["Wr1s"],
                             rhs=t_x[:, :ncols], start=False, stop=True)
            t_o = spool.tile([P, GROUP], bf16, tag="ob")
            nc.scalar.activation(out=t_o[:, :ncols], in_=ps2[:, :ncols],
                                 func=mybir.ActivationFunctionType.Lrelu,
                                 bias=w_b1s, alpha=0.01)
            transpose_out(t_o, 0, ncols, bn_ns1, g * GROUP)
        if STAGE >= 2:
            nc.gpsimd.collective_compute(
                "AllGather", mybir.AluOpType.bypass,
                replica_groups=[list(range(NCORES))],
                ins=[bn_ns1[:].opt()], outs=[tab_s1[:].opt()])
        if STAGE < 3:
            if STAGE >= 1:
                dummy_out()
            ngb = 0

        # ---- layer 1, b-dst groups
        ngsb = len(types["sb"]["groups"])   # sb dst locals < NS//NCORES
        for g in range(ngb):
            m_bb, ncols = aggregate("bb", g, wb["bb"], tabs_b0)
            wb["bb"] += len(types["bb"]["groups"][g])
            has_sb = g < ngsb
            if has_sb:
                m_sb, ncols_sb = aggregate("sb", g, wb["sb"], tabs_s0)
                wb["sb"] += len(types["sb"]["groups"][g])
            t_x = spool.tile([P, GROUP], bf16, tag="xg")
            nc.sync.dma_start(
                t_x[:, :ncols],
                bn_xb[g * GROUP:g * GROUP + ncols, :].rearrange("r f -> f r"))
            ps2 = s2pool.tile([P, GROUP], f32, space="PSUM", tag="s2")
            nc.tensor.matmul(ps2[:, :ncols], lhsT=wmat["Wl1bb"],
                             rhs=m_bb[:, :ncols], start=True, stop=False)
            if has_sb:
                nc.tensor.matmul(ps2[:, :ncols_sb], lhsT=wmat["Wl1sb"],
                                 rhs=m_sb[:, :ncols_sb], start=False, stop=False)
            nc.tensor.matmul(ps2[:, :ncols], lhsT=wmat["Wr1b"],
                             rhs=t_x[:, :ncols], start=False, stop=True)
            t_o1 = t_keep[:, g * GROUP:g * GROUP + ncols]
            nc.scalar.activation(out=t_o1, in_=ps2[:, :ncols],
                                 func=mybir.ActivationFunctionType.Lrelu,
                                 bias=w_b1b, alpha=0.01)
            transpose_out(t_keep, g * GROUP, ncols, bn_nb1, g * GROUP)
        if STAGE >= 3 and ngb:
            nc.gpsimd.collective_compute(
                "AllGather", mybir.AluOpType.bypass,
                replica_groups=[list(range(NCORES))],
                ins=[bn_nb1[:].opt()], outs=[tab_b1[:].opt()])
        if STAGE == 5:
            nc.sync.dma_start(d_dbg_b[:], bn_nb1[:])
            nc.sync.dma_start(d_dbg_s[:], bn_ns1[:])
        if STAGE < 4 or STAGE == 5:
            ngb = 0

        # ---- layer 2 (b-dst only) + heads
        wb["bb"] = 0
        wb["sb"] = 0
        for g in range(ngb):
            m_bb, ncols = aggregate("bb", g, wb["bb"], tabs_b1)
            wb["bb"] += len(types["bb"]["groups"][g])
            has_sb = g < ngsb
            if has_sb:
                m_sb, ncols_sb = aggregate("sb", g, wb["sb"], tabs_s1)
                wb["sb"] += len(types["sb"]["groups"][g])
            ps2 = s2pool.tile([P, GROUP], f32, space="PSUM", tag="s2")
            nc.tensor.matmul(ps2[:, :ncols], lhsT=wmat["Wl2bb"],
                             rhs=m_bb[:, :ncols], start=True, stop=False)
            if has_sb:
                nc.tensor.matmul(ps2[:, :ncols_sb], lhsT=wmat["Wl2sb"],
                                 rhs=m_sb[:, :ncols_sb], start=False, stop=False)
            nc.tensor.matmul(ps2[:, :ncols], lhsT=wmat["Wr2b"],
                             rhs=t_keep[:, g * GROUP:g * GROUP + ncols],
                             start=False, stop=True)
            t_o = spool.tile([P, GROUP], bf16, tag="ob")
            nc.scalar.activation(out=t_o[:, :ncols], in_=ps2[:, :ncols],
                                 func=mybir.ActivationFunctionType.Lrelu,
                                 bias=w_b2b, alpha=0.01)
            ps3 = tpool.tile([8, GROUP], f32, space="PSUM", tag="hd")
            nc.tensor.matmul(ps3[:, :ncols], lhsT=w_WhT,
                             rhs=t_o[:, :ncols], start=True, stop=True)
            t_y = spool.tile([8, GROUP], bf16, tag="yt")
            nc.vector.tensor_copy(out=t_y[:, :ncols], in_=ps3[:, :ncols])
            nc.sync.dma_start(d_yT[:, g * GROUP:g * GROUP + ncols],
                              t_y[:, :ncols])

    nc.compile()
    return nc


LAST_HW_NS = None
LAST_EXEC_S = None
LAST_PREP_S = None


def kernel(x_b, x_s, Wl, bl, Wr, Wh, bh, ei_bb, ei_sb, ei_bs):
    t_prep0 = time.time()
    x_b = np.asarray(x_b, np.float32); x_s = np.asarray(x_s, np.float32)
    Wl = np.asarray(Wl, np.float32); bl = np.asarray(bl, np.float32)
    Wr = np.asarray(Wr, np.float32); Wh = np.asarray(Wh, np.float32)
    bh = np.asarray(bh, np.float32)
    ei_bb = np.asarray(ei_bb); ei_sb = np.asarray(ei_sb); ei_bs = np.asarray(ei_bs)

    # warm the PJRT/axon connection and the jit/compile/dispatch machinery
    # outside the timed launch (one trivial 8-core program)
    import jax
    jax.device_put(np.zeros(16, np.float32), jax.devices()[0]).block_until_ready()
    nc_w = bacc.Bacc("TRN2", target_bir_lowering=False, debug=False,
                     num_devices=NCORES)
    dw_i = nc_w.dram_tensor("wi", [1, 16], mybir.dt.float32,
                            kind="ExternalInput")
    dw_o = nc_w.dram_tensor("wo", [1, 16], mybir.dt.float32,
                            kind="ExternalOutput")
    with tile.TileContext(nc_w):
        nc_w.sync.dma_start(dw_o[:], dw_i[:])
    nc_w.compile()
    try:
        run_bass_kernel_spmd(nc_w, [{"wi": np.zeros((1, 16), np.float32)}] * NCORES,
                             core_ids=list(range(NCORES)))
    except Exception:
        pass   # transient device wedge: the real launch below may still work

    def tr_b(v):
        return (v % NCORES) * NLB + v // NCORES

    def tr_s(v):
        return (v % NCORES) * NLS + v // NCORES

    # One packing shared by both layers: src ids translated to block order.
    pc_bb = _shard_edges(tr_b(ei_bb[0]), ei_bb[1], NB)
    pc_sb = _shard_edges(tr_s(ei_sb[0]), ei_sb[1], NB)   # dst b-nodes < NS
    pc_bs = _shard_edges(tr_b(ei_bs[0]), ei_bs[1], NS)
    i_bb, r_bb, v_bb, g_bb, m_bb = _pack_type(pc_bb, NLB, NB)
    i_sb, r_sb, v_sb, g_sb, m_sb = _pack_type(pc_sb, NS // NCORES, NS)
    i_bs, r_bs, v_bs, g_bs, m_bs = _pack_type(pc_bs, NLS, NB)

    cfg = {"types": {
        "bb": {"tab": "b", "Wtot": r_bb.shape[2], "groups": g_bb,
               "gb_meta": m_bb, "bcols": [a.shape[2] for a in i_bb]},
        "sb": {"tab": "s", "Wtot": r_sb.shape[2], "groups": g_sb,
               "gb_meta": m_sb, "bcols": [a.shape[2] for a in i_sb]},
        "bs": {"tab": "b", "Wtot": r_bs.shape[2], "groups": g_bs,
               "gb_meta": m_bs, "bcols": [a.shape[2] for a in i_bs]},
    }}
    nc = _build(cfg)

    # ---- pack weights
    NWB = 8 * D + 8 + S + D
    NWB += (-NWB) % 128
    wbf = np.zeros((P, NWB), BF16)
    off = 0
    for M in [Wl[0, 0], Wl[0, 1], Wr[0, 0] + Wr[0, 1], Wl[0, 2], Wr[0, 2],
              Wl[1, 0], Wl[1, 1], Wr[1, 0] + Wr[1, 1]]:
        wbf[:, off:off + D] = M.astype(BF16); off += D
    wbf[:, off:off + 8] = Wh.T.astype(BF16); off += 8
    wbf[:, off:off + S] = np.ones((1, S), BF16); off += S
    wbf[:, off:off + D] = np.eye(D, dtype=BF16); off += D
    wff = np.zeros((P, 4), np.float32)
    wff[:, 0] = bl[0, 0] + bl[0, 1]
    wff[:, 1] = bl[0, 2]
    wff[:, 2] = bl[1, 0] + bl[1, 1]
    wff[:8, 3] = bh

    xb16 = x_b.astype(BF16)
    xs16 = x_s.astype(BF16)

    # mirror _build's blob layout exactly
    NLBP = NLB + (-NLB) % GROUP
    NLSP = NLS + (-NLS) % GROUP
    iota8 = np.tile(np.arange(S, dtype=np.int8)[None, :], (P, 1))
    in_maps = []
    for c in range(NCORES):
        idx_secs = []
        for arrs in [i_bb, i_sb, i_bs]:
            for a in arrs:
                sec = a[c] if a.shape[2] >= 16 else np.zeros((16, 16), np.int16)
                idx_secs.append(sec)
        idx_all = np.concatenate(idx_secs, axis=1)
        pad = (-idx_all.shape[1]) % 128
        if pad:
            idx_all = np.concatenate(
                [idx_all, np.zeros((16, pad), np.int16)], 1)
        ivc = np.zeros(NLBP + 2 * NLSP, BF16)
        ivc[0:NLB] = v_bb[c]
        ivc[NLBP:NLBP + NLS] = v_sb[c]
        ivc[NLBP + NLSP:NLBP + NLSP + NLS] = v_bs[c]
        aux_secs = [wff.view(np.uint8)]
        ab = 16
        for r in [r_bb, r_sb, r_bs]:
            aux_secs.append(r[c].view(np.uint8))
            ab += r.shape[2]
            pad = (-ab) % 4
            if pad:
                aux_secs.append(np.zeros((P, pad), np.uint8))
                ab += pad
        aux_secs.append(iota8.view(np.uint8))
        ab += S
        pad = (-ab) % 256
        if pad:
            aux_secs.append(np.zeros((P, pad), np.uint8))
            ab += pad
        aux = np.ascontiguousarray(np.concatenate(aux_secs, axis=1))
        aux_rows = aux.reshape(-1).view(BF16).reshape(-1, D)
        idx_rows = (np.ascontiguousarray(idx_all).reshape(-1)
                    .view(BF16).reshape(-1, D))
        wb_rows = wbf[16 * c:16 * (c + 1)].reshape(-1, D)
        in_maps.append({
            "blob": np.concatenate(
                [xb16[c::NCORES], xs16[c::NCORES], ivc.reshape(-1, D),
                 wb_rows, aux_rows, idx_rows], 0),
        })
    global LAST_HW_NS, LAST_EXEC_S, LAST_PREP_S
    LAST_PREP_S = time.time() - t_prep0

    _t0 = time.time()
    res = None
    for _attempt in range(3):
        try:
            res = run_bass_kernel_spmd(nc, in_maps,
                                       core_ids=list(range(NCORES)))
            break
        except Exception:
            # NRT_EXEC_UNIT_UNRECOVERABLE wedge poisons the PJRT client for
            # the rest of the process; a fresh client (like a fresh process)
            # recovers. Tear the backends down and re-init before retrying.
            if _attempt == 2:
                raise
            try:
                import jax._src.xla_bridge as _xb
                _xb._clear_backends()
                jax.device_put(np.zeros(16, np.float32),
                               jax.devices()[0]).block_until_ready()
            except Exception:
                pass
            time.sleep(2.0)
    _exec = time.time() - _t0
    if res.exec_time_ns:
        LAST_HW_NS = int(res.exec_time_ns)
    LAST_EXEC_S = (_exec,)
    global LAST_DBG
    if "dbg_b" in res.results[0]:
        LAST_DBG = ([res.results[c]["dbg_b"] for c in range(NCORES)],
                    [res.results[c]["dbg_s"] for c in range(NCORES)])

    y = np.empty((NB, 8), np.float32)
    for c in range(NCORES):
        y[np.arange(NLB) * NCORES + c] = res.results[c]["yT"].T.astype(np.float32)
    y += bh[None, :]
    return y
